# revision 12
# baseline (speedup 1.0000x reference)
"""Trainium2 Bass kernel for nn_DeleteEdgeDecoder.

reference semantics (per batch b):
    feats[e] = [emb[i_e] | emb[j_e] | dist_e]          (513)
    h        = relu(feats @ W1 + b1)                   (E, 512)
    logits   = (h @ W2 + b2)[:, 0]  masked(-inf) + delete_bias

Sharding: pure data parallel, batch dim 128 -> 8 cores x 16.

This image's DGE only honors [128,1]-offset indirect gathers (32 calls /
batch, ~1us of serialized Pool-queue descriptor generation each — the old
bottleneck), so the embedding gather moved to host marshalling: the host
uploads feature-major featsT[b][f_lo, kt, e] = emb[end_kt(e)][f] directly
and the device program is a pure GEMM pipeline:

  - L1, edge-major PSUM: per 128-edge block, psum[e, h] accumulates
    4 K=128 matmuls (lhsT = featsT k-tile slice, rhs = W1 k-tile) plus one
    K=2 matmul [dist;1]^T @ [w_dist; b1] folding the distance feature AND
    the layer-1 bias into the same accumulation.
  - ACT: hr = relu(psum) -> fp16 SBUF.
  - L2 entirely on DVE: per block, tensor_tensor mult by broadcast w2 then
    tensor_reduce(add) along the free axis -> logits column; one per-batch
    tensor_tensor adds postbias (valid-mask -inf + b2 + delete_bias).
    (tensor_tensor_reduce would fuse these but crashes this image's DVE.)
  - One [128, 16] f32 DMA out per batch; host reassembles e = c*128+p.
"""

import os
from contextlib import ExitStack

import numpy as np
import concourse.bass as bass
import concourse.bacc as bacc
import concourse.mybir as mybir
import concourse.tile as tile
from concourse.bass_utils import run_bass_kernel_spmd

B, N, D, E = 128, 2000, 256, 2000
NCORES = 8
BL = B // NCORES          # batches per core
EP = 2048                 # edges padded
H = 512
KT = 4                    # k-tiles over [emb_i|emb_j] (4 x 128)
EB = EP // 128            # 16 edge-blocks per batch

F16 = mybir.dt.float16
F32 = mybir.dt.float32

_CACHE: dict = {}


def _build_nc(bl: int = BL):
    nc = bacc.Bacc(
        "TRN2", target_bir_lowering=False, debug=False, num_devices=NCORES
    )
    featsT = nc.dram_tensor("featsT", [bl, 128, KT * EP], F16, kind="ExternalInput")
    dist_ones = nc.dram_tensor("dist_ones", [bl, 2, EP], F16, kind="ExternalInput")
    pb = nc.dram_tensor("pb", [bl, 128, EB], F32, kind="ExternalInput")
    w1p = nc.dram_tensor("w1p", [128, KT * H], F16, kind="ExternalInput")
    wdb1 = nc.dram_tensor("wdb1", [2, H], F16, kind="ExternalInput")
    w2bc = nc.dram_tensor("w2bc", [128, H], F16, kind="ExternalInput")
    out = nc.dram_tensor("out", [bl, 128, EB], F32, kind="ExternalOutput")

    with tile.TileContext(nc) as tc, ExitStack() as ctx:
        const = ctx.enter_context(tc.tile_pool(name="const", bufs=1))
        ftp = ctx.enter_context(tc.tile_pool(name="ft", bufs=3))
        dop = ctx.enter_context(tc.tile_pool(name="do", bufs=2))
        pbp = ctx.enter_context(tc.tile_pool(name="pb", bufs=2))
        hrp = ctx.enter_context(tc.tile_pool(name="hr", bufs=4))
        ttp = ctx.enter_context(tc.tile_pool(name="tt", bufs=3))
        lap = ctx.enter_context(tc.tile_pool(name="la", bufs=2))
        psp = ctx.enter_context(tc.tile_pool(name="ps", bufs=8, space="PSUM"))

        w1_sb = const.tile([128, KT, H], F16)
        nc.sync.dma_start(w1_sb[:], w1p.ap())
        wdb1_sb = const.tile([2, H], F16)
        nc.sync.dma_start(wdb1_sb[:], wdb1.ap())
        w2bc_sb = const.tile([128, H], F16)
        nc.sync.dma_start(w2bc_sb[:], w2bc.ap())

        for b in range(bl):
            ft = ftp.tile([128, KT, EP], F16, tag="ft")
            for kt in range(KT):
                nc.sync.dma_start(
                    ft[:, kt, :], featsT.ap()[b, :, kt * EP : (kt + 1) * EP]
                )
            do = dop.tile([2, EP], F16, tag="do")
            nc.gpsimd.dma_start(do[:], dist_ones.ap()[b])
            pbt = pbp.tile([128, EB], F32, tag="pb")
            nc.gpsimd.dma_start(pbt[:], pb.ap()[b])

            la = lap.tile([128, EB], F32, tag="la")
            for eb in range(EB):
                es = slice(eb * 128, (eb + 1) * 128)
                ph = psp.tile([128, H], F32, tag="ps")
                for kt in range(KT):
                    nc.tensor.matmul(
                        ph[:],
                        ft[:, kt, es],
                        w1_sb[:, kt, :],
                        start=(kt == 0),
                        stop=False,
                    )
                # += [dist;1]^T @ [w_dist; b1]  (K=2 rank-2 update)
                nc.tensor.matmul(
                    ph[:],
                    do[:, es],
                    wdb1_sb[:],
                    start=False,
                    stop=True,
                )
                hr = hrp.tile([128, H], F16, tag="hr")
                nc.scalar.activation(
                    hr[:], ph[:], mybir.ActivationFunctionType.Relu
                )
                tt = ttp.tile([128, H], F16, tag="tt")
                nc.vector.tensor_tensor(
                    tt[:], hr[:], w2bc_sb[:], mybir.AluOpType.mult
                )
                nc.vector.tensor_reduce(
                    la[:, eb : eb + 1], tt[:], mybir.AxisListType.X,
                    mybir.AluOpType.add,
                )
            nc.vector.tensor_tensor(la[:], la[:], pbt[:], mybir.AluOpType.add)
            nc.gpsimd.dma_start(out.ap()[b], la[:])

    nc.compile()
    return nc


def _prep_core_inputs(core, node_embeddings, locs, edge_list, delete_bias,
                      W1, b1, W2, b2, bl: int = BL):
    """Build the per-core input map (layout/dtype marshalling + host gather)."""
    b0 = core * bl
    emb16 = node_embeddings[b0 : b0 + bl].astype(np.float16)  # (bl, N, D)

    el = edge_list[b0 : b0 + bl]  # (bl, E, 2) int
    iclip = np.maximum(el[..., 0], 0).astype(np.int64)
    jclip = np.maximum(el[..., 1], 0).astype(np.int64)
    ipad = np.zeros((bl, EP), dtype=np.int64)
    ipad[:, :E] = iclip
    jpad = np.zeros((bl, EP), dtype=np.int64)
    jpad[:, :E] = jclip

    bidx = np.arange(bl)[:, None]
    gi = emb16[bidx, ipad]  # (bl, EP, D)
    gj = emb16[bidx, jpad]
    fti = gi.reshape(bl, EP, 2, 128).transpose(0, 3, 2, 1)  # (bl,128,2,EP)
    ftj = gj.reshape(bl, EP, 2, 128).transpose(0, 3, 2, 1)
    featsT = np.ascontiguousarray(
        np.concatenate([fti, ftj], axis=2).reshape(bl, 128, KT * EP)
    )

    lc = locs[b0 : b0 + bl]
    dvec = lc[bidx, iclip] - lc[bidx, jclip]
    dist = np.sqrt((dvec * dvec).sum(-1)).astype(np.float16)  # (bl, E)
    dist_ones = np.ones((bl, 2, EP), dtype=np.float16)
    dist_ones[:, 0, :] = 0.0
    dist_ones[:, 0, :E] = dist

    valid = (el[..., 0] >= 0) & (el[..., 1] >= 0)
    pbfull = np.zeros((bl, EP), dtype=np.float32)
    pbfull[:, :E] = (
        np.where(valid, 0.0, -np.inf)
        + float(np.asarray(b2).reshape(-1)[0])
        + float(delete_bias)
    )
    pb = np.ascontiguousarray(
        pbfull.reshape(bl, EB, 128).transpose(0, 2, 1)
    )  # pb[b, p, c] = pbfull[b, c*128+p]

    w1p = np.ascontiguousarray(
        W1[: 2 * D].reshape(KT, 128, H).transpose(1, 0, 2).reshape(128, KT * H)
    ).astype(np.float16)
    wdb1 = np.stack([W1[2 * D], b1]).astype(np.float16)  # (2, H)
    w2bc = np.ascontiguousarray(
        np.broadcast_to(W2[:, 0].astype(np.float16), (128, H))
    )

    return {
        "featsT": featsT,
        "dist_ones": dist_ones,
        "pb": pb,
        "w1p": w1p,
        "wdb1": wdb1,
        "w2bc": w2bc,
    }


def kernel(node_embeddings, locs, edge_list, delete_bias, W1, b1, W2, b2):
    node_embeddings = np.asarray(node_embeddings, dtype=np.float32)
    locs = np.asarray(locs, dtype=np.float32)
    edge_list = np.asarray(edge_list)
    W1 = np.asarray(W1, dtype=np.float32)
    b1 = np.asarray(b1, dtype=np.float32)
    W2 = np.asarray(W2, dtype=np.float32)
    b2 = np.asarray(b2, dtype=np.float32)

    if "nc" not in _CACHE:
        _CACHE["nc"] = _build_nc()
    nc = _CACHE["nc"]

    in_maps = [
        _prep_core_inputs(c, node_embeddings, locs, edge_list, delete_bias,
                          W1, b1, W2, b2)
        for c in range(NCORES)
    ]
    trace = os.environ.get("BASS_KERNEL_TRACE", "0") == "1"
    res = run_bass_kernel_spmd(nc, in_maps, list(range(NCORES)), trace=trace)
    _CACHE["last_result"] = res

    outs = []
    for c in range(NCORES):
        o = np.asarray(res.results[c]["out"], dtype=np.float32)  # (bl,128,EB)
        o = o.transpose(0, 2, 1).reshape(BL, EP)  # e = c*128 + p
        outs.append(o[:, :E])
    return np.concatenate(outs, axis=0)


# revision 17
# speedup vs baseline: 1.1910x; 1.1910x over previous
"""Trainium2 Bass kernel for nn_DeleteEdgeDecoder.

reference semantics (per batch b):
    feats[e] = [emb[i_e] | emb[j_e] | dist_e]          (513)
    h        = relu(feats @ W1 + b1)                   (E, 512)
    logits   = (h @ W2 + b2)[:, 0]  masked(-inf) + delete_bias

Sharding: pure data parallel, batch dim 128 -> 8 cores x 16.

This image's DGE only honors [128,1]-offset indirect gathers (32 calls /
batch, ~1us of serialized Pool-queue descriptor generation each — the old
bottleneck), so the embedding gather moved to host marshalling: the host
uploads feature-major featsT[b][f_lo, kt, e] = emb[end_kt(e)][f] directly
and the device program is a pure GEMM pipeline:

  - L1, edge-major PSUM: per 128-edge block, psum[e, h] accumulates
    4 K=128 matmuls (lhsT = featsT k-tile slice, rhs = W1 k-tile) plus one
    K=2 matmul [dist;1]^T @ [w_dist; b1] folding the distance feature AND
    the layer-1 bias into the same accumulation.
  - ACT: hr = relu(psum) -> fp16 SBUF.
  - L2 entirely on DVE: per block, tensor_tensor mult by broadcast w2 then
    tensor_reduce(add) along the free axis -> logits column; one per-batch
    tensor_tensor adds postbias (valid-mask -inf + b2 + delete_bias).
    (tensor_tensor_reduce would fuse these but crashes this image's DVE.)
  - One [128, 16] f32 DMA out per batch; host reassembles e = c*128+p.
"""

import os
from contextlib import ExitStack

import numpy as np
import concourse.bass as bass
import concourse.bacc as bacc
import concourse.mybir as mybir
import concourse.tile as tile
from concourse.bass_utils import run_bass_kernel_spmd

B, N, D, E = 128, 2000, 256, 2000
NCORES = 8
BL = B // NCORES          # batches per core
EP = 2048                 # edges padded
H = 512
KT = 4                    # k-tiles over [emb_i|emb_j] (4 x 128)
EB = EP // 128            # 16 edge-blocks per batch

F16 = mybir.dt.float16
F32 = mybir.dt.float32

_CACHE: dict = {}


def _build_nc(bl: int = BL):
    nc = bacc.Bacc(
        "TRN2", target_bir_lowering=False, debug=False, num_devices=NCORES
    )
    featsT = nc.dram_tensor("featsT", [bl, 128, KT * EP], F16, kind="ExternalInput")
    dist_ones = nc.dram_tensor("dist_ones", [bl, 2, EP], F16, kind="ExternalInput")
    pb = nc.dram_tensor("pb", [bl, 128, EB], F32, kind="ExternalInput")
    w1p = nc.dram_tensor("w1p", [128, KT * H], F16, kind="ExternalInput")
    wdb1 = nc.dram_tensor("wdb1", [2, H], F16, kind="ExternalInput")
    w2bc = nc.dram_tensor("w2bc", [128, H], F16, kind="ExternalInput")
    out = nc.dram_tensor("out", [bl, 128, EB], F32, kind="ExternalOutput")

    with tile.TileContext(nc) as tc, ExitStack() as ctx:
        const = ctx.enter_context(tc.tile_pool(name="const", bufs=1))
        ftp = ctx.enter_context(tc.tile_pool(name="ft", bufs=3))
        dop = ctx.enter_context(tc.tile_pool(name="do", bufs=2))
        pbp = ctx.enter_context(tc.tile_pool(name="pb", bufs=2))
        hrp = ctx.enter_context(tc.tile_pool(name="hr", bufs=4))
        ttp = ctx.enter_context(tc.tile_pool(name="tt", bufs=3))
        lap = ctx.enter_context(tc.tile_pool(name="la", bufs=2))
        psp = ctx.enter_context(tc.tile_pool(name="ps", bufs=7, space="PSUM"))
        wpsp = ctx.enter_context(tc.tile_pool(name="wps", bufs=1, space="PSUM"))

        # PE clock pre-warm: the Tensor engine ramps to full clock only
        # after ~3us of continuous execution. A chain of tiny self-contained
        # matmuls on a zeroed tile keeps PE busy through the initial DMA fill
        # so every real matmul runs at full clock from the start.
        warm = const.tile([128, 64], F16)
        nc.vector.memset(warm[:], 0.0)
        wps = wpsp.tile([64, 64], F32, tag="warm")
        for i in range(116):
            nc.tensor.matmul(wps[:], warm[:, :], warm[:, :],
                             start=True, stop=True)

        w1_sb = const.tile([128, KT, H], F16)
        nc.sync.dma_start(w1_sb[:], w1p.ap())
        wdb1_sb = const.tile([2, H], F16)
        nc.gpsimd.dma_start(wdb1_sb[:], wdb1.ap())
        w2bc_sb = const.tile([128, H], F16)
        nc.gpsimd.dma_start(w2bc_sb[:], w2bc.ap())

        for b in range(bl):
            ft = ftp.tile([128, KT, EP], F16, tag="ft")
            for kt in range(KT):
                nc.sync.dma_start(
                    ft[:, kt, :], featsT.ap()[b, :, kt * EP : (kt + 1) * EP]
                )
            do = dop.tile([2, EP], F16, tag="do")
            nc.gpsimd.dma_start(do[:], dist_ones.ap()[b])
            pbt = pbp.tile([128, EB], F32, tag="pb")
            nc.gpsimd.dma_start(pbt[:], pb.ap()[b])

            la = lap.tile([128, EB], F32, tag="la")
            for eb in range(EB):
                es = slice(eb * 128, (eb + 1) * 128)
                ph = psp.tile([128, H], F32, tag="ps")
                for kt in range(KT):
                    nc.tensor.matmul(
                        ph[:],
                        ft[:, kt, es],
                        w1_sb[:, kt, :],
                        start=(kt == 0),
                        stop=False,
                    )
                # += [dist;1]^T @ [w_dist; b1]  (K=2 rank-2 update)
                nc.tensor.matmul(
                    ph[:],
                    do[:, es],
                    wdb1_sb[:],
                    start=False,
                    stop=True,
                )
                hr = hrp.tile([128, H], F16, tag="hr")
                nc.scalar.activation(
                    hr[:], ph[:], mybir.ActivationFunctionType.Relu
                )
                tt = ttp.tile([128, H], F16, tag="tt")
                nc.vector.tensor_tensor(
                    tt[:], hr[:], w2bc_sb[:], mybir.AluOpType.mult
                )
                nc.vector.tensor_reduce(
                    la[:, eb : eb + 1], tt[:], mybir.AxisListType.X,
                    mybir.AluOpType.add,
                )
            nc.vector.tensor_tensor(la[:], la[:], pbt[:], mybir.AluOpType.add)
            nc.gpsimd.dma_start(out.ap()[b], la[:])

    nc.compile()
    return nc


def _prep_core_inputs(core, node_embeddings, locs, edge_list, delete_bias,
                      W1, b1, W2, b2, bl: int = BL):
    """Build the per-core input map (layout/dtype marshalling + host gather)."""
    b0 = core * bl
    emb16 = node_embeddings[b0 : b0 + bl].astype(np.float16)  # (bl, N, D)

    el = edge_list[b0 : b0 + bl]  # (bl, E, 2) int
    iclip = np.maximum(el[..., 0], 0).astype(np.int64)
    jclip = np.maximum(el[..., 1], 0).astype(np.int64)
    ipad = np.zeros((bl, EP), dtype=np.int64)
    ipad[:, :E] = iclip
    jpad = np.zeros((bl, EP), dtype=np.int64)
    jpad[:, :E] = jclip

    bidx = np.arange(bl)[:, None]
    gi = emb16[bidx, ipad]  # (bl, EP, D)
    gj = emb16[bidx, jpad]
    fti = gi.reshape(bl, EP, 2, 128).transpose(0, 3, 2, 1)  # (bl,128,2,EP)
    ftj = gj.reshape(bl, EP, 2, 128).transpose(0, 3, 2, 1)
    featsT = np.ascontiguousarray(
        np.concatenate([fti, ftj], axis=2).reshape(bl, 128, KT * EP)
    )

    lc = locs[b0 : b0 + bl]
    dvec = lc[bidx, iclip] - lc[bidx, jclip]
    dist = np.sqrt((dvec * dvec).sum(-1)).astype(np.float16)  # (bl, E)
    dist_ones = np.ones((bl, 2, EP), dtype=np.float16)
    dist_ones[:, 0, :] = 0.0
    dist_ones[:, 0, :E] = dist

    valid = (el[..., 0] >= 0) & (el[..., 1] >= 0)
    pbfull = np.zeros((bl, EP), dtype=np.float32)
    pbfull[:, :E] = (
        np.where(valid, 0.0, -np.inf)
        + float(np.asarray(b2).reshape(-1)[0])
        + float(delete_bias)
    )
    pb = np.ascontiguousarray(
        pbfull.reshape(bl, EB, 128).transpose(0, 2, 1)
    )  # pb[b, p, c] = pbfull[b, c*128+p]

    w1p = np.ascontiguousarray(
        W1[: 2 * D].reshape(KT, 128, H).transpose(1, 0, 2).reshape(128, KT * H)
    ).astype(np.float16)
    wdb1 = np.stack([W1[2 * D], b1]).astype(np.float16)  # (2, H)
    w2bc = np.ascontiguousarray(
        np.broadcast_to(W2[:, 0].astype(np.float16), (128, H))
    )

    return {
        "featsT": featsT,
        "dist_ones": dist_ones,
        "pb": pb,
        "w1p": w1p,
        "wdb1": wdb1,
        "w2bc": w2bc,
    }


def kernel(node_embeddings, locs, edge_list, delete_bias, W1, b1, W2, b2):
    node_embeddings = np.asarray(node_embeddings, dtype=np.float32)
    locs = np.asarray(locs, dtype=np.float32)
    edge_list = np.asarray(edge_list)
    W1 = np.asarray(W1, dtype=np.float32)
    b1 = np.asarray(b1, dtype=np.float32)
    W2 = np.asarray(W2, dtype=np.float32)
    b2 = np.asarray(b2, dtype=np.float32)

    if "nc" not in _CACHE:
        _CACHE["nc"] = _build_nc()
    nc = _CACHE["nc"]

    in_maps = [
        _prep_core_inputs(c, node_embeddings, locs, edge_list, delete_bias,
                          W1, b1, W2, b2)
        for c in range(NCORES)
    ]
    trace = os.environ.get("BASS_KERNEL_TRACE", "0") == "1"
    res = run_bass_kernel_spmd(nc, in_maps, list(range(NCORES)), trace=trace)
    _CACHE["last_result"] = res

    outs = []
    for c in range(NCORES):
        o = np.asarray(res.results[c]["out"], dtype=np.float32)  # (bl,128,EB)
        o = o.transpose(0, 2, 1).reshape(BL, EP)  # e = c*128 + p
        outs.append(o[:, :E])
    return np.concatenate(outs, axis=0)


# revision 18
# speedup vs baseline: 1.1910x; 1.0000x over previous
"""Trainium2 Bass kernel for nn_DeleteEdgeDecoder.

reference semantics (per batch b):
    feats[e] = [emb[i_e] | emb[j_e] | dist_e]          (513)
    h        = relu(feats @ W1 + b1)                   (E, 512)
    logits   = (h @ W2 + b2)[:, 0]  masked(-inf) + delete_bias

Sharding: pure data parallel, batch dim 128 -> 8 cores x 16.

This image's DGE only honors [128,1]-offset indirect gathers (32 calls /
batch, ~1us of serialized Pool-queue descriptor generation each — the old
bottleneck), so the embedding gather moved to host marshalling: the host
uploads feature-major featsT[b][f_lo, kt, e] = emb[end_kt(e)][f] directly
and the device program is a pure GEMM pipeline:

  - L1, edge-major PSUM: per 128-edge block, psum[e, h] accumulates
    4 K=128 matmuls (lhsT = featsT k-tile slice, rhs = W1 k-tile) plus one
    K=2 matmul [dist;1]^T @ [w_dist; b1] folding the distance feature AND
    the layer-1 bias into the same accumulation.
  - ACT: hr = relu(psum) -> fp16 SBUF.
  - L2 entirely on DVE: per block, tensor_tensor mult by broadcast w2 then
    tensor_reduce(add) along the free axis -> logits column; one per-batch
    tensor_tensor adds postbias (valid-mask -inf + b2 + delete_bias).
    (tensor_tensor_reduce would fuse these but crashes this image's DVE.)
  - One [128, 16] f32 DMA out per batch; host reassembles e = c*128+p.
"""

import os
from contextlib import ExitStack

import numpy as np
import concourse.bass as bass
import concourse.bacc as bacc
import concourse.mybir as mybir
import concourse.tile as tile
from concourse.bass_utils import run_bass_kernel_spmd

B, N, D, E = 128, 2000, 256, 2000
NCORES = 8
BL = B // NCORES          # batches per core
EP = 2048                 # edges padded
H = 512
KT = 4                    # k-tiles over [emb_i|emb_j] (4 x 128)
EB = EP // 128            # 16 edge-blocks per batch

F16 = mybir.dt.float16
F32 = mybir.dt.float32

_CACHE: dict = {}


def _build_nc(bl: int = BL):
    nc = bacc.Bacc(
        "TRN2", target_bir_lowering=False, debug=False, num_devices=NCORES
    )
    featsT = nc.dram_tensor("featsT", [bl, 128, KT * EP], F16, kind="ExternalInput")
    dist_ones = nc.dram_tensor("dist_ones", [bl, 2, EP], F16, kind="ExternalInput")
    pb = nc.dram_tensor("pb", [bl, 128, EB], F32, kind="ExternalInput")
    w1p = nc.dram_tensor("w1p", [128, KT * H], F16, kind="ExternalInput")
    wdb1 = nc.dram_tensor("wdb1", [2, H], F16, kind="ExternalInput")
    w2bc = nc.dram_tensor("w2bc", [128, H], F16, kind="ExternalInput")
    out = nc.dram_tensor("out", [bl, 128, EB], F32, kind="ExternalOutput")

    with tile.TileContext(nc) as tc, ExitStack() as ctx:
        const = ctx.enter_context(tc.tile_pool(name="const", bufs=1))
        ftp = ctx.enter_context(tc.tile_pool(name="ft", bufs=3))
        dop = ctx.enter_context(tc.tile_pool(name="do", bufs=2))
        pbp = ctx.enter_context(tc.tile_pool(name="pb", bufs=2))
        hrp = ctx.enter_context(tc.tile_pool(name="hr", bufs=4))
        ttp = ctx.enter_context(tc.tile_pool(name="tt", bufs=3))
        lap = ctx.enter_context(tc.tile_pool(name="la", bufs=2))
        psp = ctx.enter_context(tc.tile_pool(name="ps", bufs=7, space="PSUM"))
        wpsp = ctx.enter_context(tc.tile_pool(name="wps", bufs=1, space="PSUM"))

        # PE clock pre-warm: the Tensor engine ramps to full clock only
        # after ~3us of continuous execution. A chain of tiny self-contained
        # matmuls on a zeroed tile keeps PE busy through the initial DMA fill
        # so every real matmul runs at full clock from the start.
        warm = const.tile([128, 64], F16)
        nc.vector.memset(warm[:], 0.0)
        wps = wpsp.tile([64, 64], F32, tag="warm")
        for i in range(116):
            nc.tensor.matmul(wps[:], warm[:, :], warm[:, :],
                             start=True, stop=True)

        w1_sb = const.tile([128, KT, H], F16)
        nc.sync.dma_start(w1_sb[:], w1p.ap())
        wdb1_sb = const.tile([2, H], F16)
        nc.gpsimd.dma_start(wdb1_sb[:], wdb1.ap())
        w2bc_sb = const.tile([128, H], F16)
        nc.gpsimd.dma_start(w2bc_sb[:], w2bc.ap())

        for b in range(bl):
            ft = ftp.tile([128, KT, EP], F16, tag="ft")
            for kt in range(KT):
                nc.sync.dma_start(
                    ft[:, kt, :], featsT.ap()[b, :, kt * EP : (kt + 1) * EP]
                )
            do = dop.tile([2, EP], F16, tag="do")
            nc.gpsimd.dma_start(do[:], dist_ones.ap()[b])
            pbt = pbp.tile([128, EB], F32, tag="pb")
            nc.gpsimd.dma_start(pbt[:], pb.ap()[b])

            la = lap.tile([128, EB], F32, tag="la")
            for eb in range(EB):
                es = slice(eb * 128, (eb + 1) * 128)
                ph = psp.tile([128, H], F32, tag="ps")
                for kt in range(KT):
                    nc.tensor.matmul(
                        ph[:],
                        ft[:, kt, es],
                        w1_sb[:, kt, :],
                        start=(kt == 0),
                        stop=False,
                    )
                # += [dist;1]^T @ [w_dist; b1]  (K=2 rank-2 update)
                nc.tensor.matmul(
                    ph[:],
                    do[:, es],
                    wdb1_sb[:],
                    start=False,
                    stop=True,
                )
                hr = hrp.tile([128, H], F16, tag="hr")
                nc.scalar.activation(
                    hr[:], ph[:], mybir.ActivationFunctionType.Relu
                )
                tt = ttp.tile([128, H], F16, tag="tt")
                nc.vector.tensor_tensor(
                    tt[:], hr[:], w2bc_sb[:], mybir.AluOpType.mult
                )
                nc.vector.tensor_reduce(
                    la[:, eb : eb + 1], tt[:], mybir.AxisListType.X,
                    mybir.AluOpType.add,
                )
                if eb == EB // 2 - 1:
                    nc.vector.tensor_tensor(
                        la[:, : EB // 2], la[:, : EB // 2],
                        pbt[:, : EB // 2], mybir.AluOpType.add,
                    )
                    nc.gpsimd.dma_start(
                        out.ap()[b][:, : EB // 2], la[:, : EB // 2]
                    )
            nc.vector.tensor_tensor(
                la[:, EB // 2 :], la[:, EB // 2 :],
                pbt[:, EB // 2 :], mybir.AluOpType.add,
            )
            nc.gpsimd.dma_start(out.ap()[b][:, EB // 2 :], la[:, EB // 2 :])

    nc.compile()
    return nc


def _prep_core_inputs(core, node_embeddings, locs, edge_list, delete_bias,
                      W1, b1, W2, b2, bl: int = BL):
    """Build the per-core input map (layout/dtype marshalling + host gather)."""
    b0 = core * bl
    emb16 = node_embeddings[b0 : b0 + bl].astype(np.float16)  # (bl, N, D)

    el = edge_list[b0 : b0 + bl]  # (bl, E, 2) int
    iclip = np.maximum(el[..., 0], 0).astype(np.int64)
    jclip = np.maximum(el[..., 1], 0).astype(np.int64)
    ipad = np.zeros((bl, EP), dtype=np.int64)
    ipad[:, :E] = iclip
    jpad = np.zeros((bl, EP), dtype=np.int64)
    jpad[:, :E] = jclip

    bidx = np.arange(bl)[:, None]
    gi = emb16[bidx, ipad]  # (bl, EP, D)
    gj = emb16[bidx, jpad]
    fti = gi.reshape(bl, EP, 2, 128).transpose(0, 3, 2, 1)  # (bl,128,2,EP)
    ftj = gj.reshape(bl, EP, 2, 128).transpose(0, 3, 2, 1)
    featsT = np.ascontiguousarray(
        np.concatenate([fti, ftj], axis=2).reshape(bl, 128, KT * EP)
    )

    lc = locs[b0 : b0 + bl]
    dvec = lc[bidx, iclip] - lc[bidx, jclip]
    dist = np.sqrt((dvec * dvec).sum(-1)).astype(np.float16)  # (bl, E)
    dist_ones = np.ones((bl, 2, EP), dtype=np.float16)
    dist_ones[:, 0, :] = 0.0
    dist_ones[:, 0, :E] = dist

    valid = (el[..., 0] >= 0) & (el[..., 1] >= 0)
    pbfull = np.zeros((bl, EP), dtype=np.float32)
    pbfull[:, :E] = (
        np.where(valid, 0.0, -np.inf)
        + float(np.asarray(b2).reshape(-1)[0])
        + float(delete_bias)
    )
    pb = np.ascontiguousarray(
        pbfull.reshape(bl, EB, 128).transpose(0, 2, 1)
    )  # pb[b, p, c] = pbfull[b, c*128+p]

    w1p = np.ascontiguousarray(
        W1[: 2 * D].reshape(KT, 128, H).transpose(1, 0, 2).reshape(128, KT * H)
    ).astype(np.float16)
    wdb1 = np.stack([W1[2 * D], b1]).astype(np.float16)  # (2, H)
    w2bc = np.ascontiguousarray(
        np.broadcast_to(W2[:, 0].astype(np.float16), (128, H))
    )

    return {
        "featsT": featsT,
        "dist_ones": dist_ones,
        "pb": pb,
        "w1p": w1p,
        "wdb1": wdb1,
        "w2bc": w2bc,
    }


def kernel(node_embeddings, locs, edge_list, delete_bias, W1, b1, W2, b2):
    node_embeddings = np.asarray(node_embeddings, dtype=np.float32)
    locs = np.asarray(locs, dtype=np.float32)
    edge_list = np.asarray(edge_list)
    W1 = np.asarray(W1, dtype=np.float32)
    b1 = np.asarray(b1, dtype=np.float32)
    W2 = np.asarray(W2, dtype=np.float32)
    b2 = np.asarray(b2, dtype=np.float32)

    if "nc" not in _CACHE:
        _CACHE["nc"] = _build_nc()
    nc = _CACHE["nc"]

    in_maps = [
        _prep_core_inputs(c, node_embeddings, locs, edge_list, delete_bias,
                          W1, b1, W2, b2)
        for c in range(NCORES)
    ]
    trace = os.environ.get("BASS_KERNEL_TRACE", "0") == "1"
    res = run_bass_kernel_spmd(nc, in_maps, list(range(NCORES)), trace=trace)
    _CACHE["last_result"] = res

    outs = []
    for c in range(NCORES):
        o = np.asarray(res.results[c]["out"], dtype=np.float32)  # (bl,128,EB)
        o = o.transpose(0, 2, 1).reshape(BL, EP)  # e = c*128 + p
        outs.append(o[:, :E])
    return np.concatenate(outs, axis=0)
